# revision 33
# baseline (speedup 1.0000x reference)
"""Trainium2 Bass kernel for nn_AeloruLayer (Hi-DoRA/Fisher-gated LoRA linear).

Computation (reference semantics, all fp32):
    dw     = (alpha/r) * (lora_B @ lora_A) * m[:, None] / (1 + gamma * fisher)
    factor = min(1, eta * ||W0||_F / max(||dw||_F, eps))   (branchless-equivalent)
    W_eff  = W0 + W_acc + factor * dw
    y      = x @ W_eff^T + bias

Sharding across 8 NeuronCores: 2-way over batch/tokens x 4-way over
out_features.  Each core computes a (TOK x DOUT) block of y; the host
concatenates.  The Frobenius norms need a global reduction: each core
reduces its shard, then a tiny AllReduce over the 4 cores that together
hold the full weight matrix (one batch group) combines them.

Device pipeline per core:
  Phase 1 (weights): stream W0/W_acc/fisher shard in [128 x IN_CHUNK]
    tiles; dw = (B*m @ A) / (1+gamma*F) via PE matmul (K=16) + DVE
    reciprocal/multiply; accumulate sum(dw^2), sum(W0^2) on ACT;
    S = W0+W_acc; PE-transpose S into the SBUF-resident W_eff^T buffer;
    PE-transpose dw and spill dw^T to DRAM (SBUF can't hold both S^T
    and dw^T).
  Sync: AllReduce([sum_dw2, sum_w02]) -> factor (branchless min/sqrt),
    broadcast to all partitions via a K=1 ones-matmul.
  Combine: stream dw^T back, W_eff^T += factor * dw^T.
  Phase 2 (tokens): per 128-token tile, PE-transpose x into x^T
    (contraction dim must sit on partitions), then float32r matmuls
    accumulate y tiles in PSUM; evict with fused bias add; DMA out.
"""

from contextlib import ExitStack

import numpy as np

import concourse.bass as bass
import concourse.mybir as mybir
import concourse.tile as tile
from concourse import bacc
from concourse.bass import ds, ts
from concourse.bass_utils import run_bass_kernel_spmd
from concourse.masks import make_identity

F32 = mybir.dt.float32
F32R = mybir.dt.float32r
P = 128

LORA_ALPHA = 16.0
R_RANK = 16
FISHER_GAMMA = 10.0
ENERGY_ETA = 0.15

# problem shapes (full)
B_FULL, S_FULL, D_IN, D_OUT = 4, 2048, 4096, 4096
N_CORES = 8
ROW_GROUPS = 2  # batch split
COL_GROUPS = 4  # out_features split


class Cfg:
    def __init__(self, tok, din, dout, rank=R_RANK, n_cores=N_CORES,
                 row_groups=ROW_GROUPS, col_groups=COL_GROUPS,
                 in_chunk=1024, nfree=512, use_f32r=True, probe=()):
        self.tok = tok          # tokens per core
        self.din = din          # contraction dim (full)
        self.dout = dout        # out features per core
        self.rank = rank
        self.n_cores = n_cores
        self.row_groups = row_groups
        self.col_groups = col_groups
        self.in_chunk = min(in_chunk, din)
        self.nfree = min(nfree, dout)
        self.use_f32r = use_f32r
        self.probe = set(probe)  # timing-probe ablations, see build_nc
        assert tok % P == 0 and din % P == 0 and dout % P == 0
        assert din % self.in_chunk == 0 and self.in_chunk % P == 0
        assert dout % self.nfree == 0


def build_nc(cfg: Cfg, variant="full"):
    """variant: "full" | "ph1" (weights pipeline only) | "ph2" (token
    matmul pipeline only, wt from memset) | "nocc" (full, collective
    replaced by a local DMA copy). Non-full variants are for timing
    bisection only."""
    nc = bacc.Bacc("TRN2", target_bir_lowering=False, debug=False,
                   num_devices=cfg.n_cores)
    tok, din, dout = cfg.tok, cfg.din, cfg.dout
    KB = din // P            # 128-blocks along contraction dim
    OB = dout // P           # 128-blocks along out dim
    IC = din // cfg.in_chunk
    CPB = cfg.in_chunk // P  # 128-blocks per phase-1 chunk
    MT = tok // P            # token tiles
    NT = dout // cfg.nfree   # out tiles in phase 2
    mm_dt = F32R if cfg.use_f32r else F32

    x_d = nc.dram_tensor("x", [tok, din], F32, kind="ExternalInput").ap()
    w0_d = nc.dram_tensor("w0", [dout, din], F32, kind="ExternalInput").ap()
    wa_d = nc.dram_tensor("wacc", [dout, din], F32, kind="ExternalInput").ap()
    fi_d = nc.dram_tensor("fisher", [dout, din], F32, kind="ExternalInput").ap()
    la_d = nc.dram_tensor("lora_a", [cfg.rank, din], F32, kind="ExternalInput").ap()
    lb_d = nc.dram_tensor("lora_b", [dout, cfg.rank], F32, kind="ExternalInput").ap()
    m_d = nc.dram_tensor("mvec", [dout], F32, kind="ExternalInput").ap()
    bias_d = nc.dram_tensor("bias", [dout], F32, kind="ExternalInput").ap()
    y_d = nc.dram_tensor("y", [tok, dout], F32, kind="ExternalOutput").ap()
    BF16 = mybir.dt.bfloat16
    dwn_d = nc.dram_tensor("dw_spill", [dout, din], BF16).ap()
    cc_in_d = nc.dram_tensor("cc_in", [2], F32).ap()
    cc_out_d = nc.dram_tensor("cc_out", [2], F32).ap()

    # out-block views: row (ob*128 + p) -> [p, ob, :]
    w0_v = w0_d.rearrange("(ob p) i -> p ob i", p=P)
    wa_v = wa_d.rearrange("(ob p) i -> p ob i", p=P)
    fi_v = fi_d.rearrange("(ob p) i -> p ob i", p=P)
    dwn_v = dwn_d.rearrange("(ob p) i -> p ob i", p=P)

    groups = [[i * cfg.col_groups + j for j in range(cfg.col_groups)]
              for i in range(cfg.row_groups)]

    with tile.TileContext(nc) as tc, ExitStack() as ctx:
        const = ctx.enter_context(tc.tile_pool(name="const", bufs=1))
        identity = const.tile([P, P], F32)
        make_identity(nc, identity)
        ones_full = const.tile([P, P], F32)
        nc.vector.memset(ones_full[:], 1.0)
        ones_1 = const.tile([1, P], F32)
        nc.vector.memset(ones_1[:], 1.0)
        identity_bf = const.tile([P, P], BF16)
        make_identity(nc, identity_bf)
        identity_r = const.tile([P, P], F32R)
        nc.vector.tensor_copy(identity_r[:], identity[:])
        stats_dw = const.tile([P, OB * IC], F32)
        nc.vector.memset(stats_dw[:], 0.0)
        stats_w0 = const.tile([P, OB * IC], F32)
        nc.vector.memset(stats_w0[:], 0.0)
        stats2 = const.tile([P, 2], F32)
        tot_bc = const.tile([P, 2], F32)
        fac = const.tile([P, 1], F32)
        bias_bc = const.tile([P, dout], F32)
        t1 = const.tile([1, 2], F32)
        flagi = const.tile([P, 1], mybir.dt.int32)
        facm1 = const.tile([P, 1], F32)

        # W_eff^T, resident: [in-part, in-block, out].  Typed float32r so
        # every write rounds, as the fp32r matmul verifier requires.
        wt = const.tile([P, KB, dout], mm_dt, name="w_eff_t")

        psum_mm = ctx.enter_context(
            tc.tile_pool(name="psum_mm",
                         bufs=3 if "mm3" in cfg.probe else 2, space="PSUM"))
        psum_tr = ctx.enter_context(
            tc.tile_pool(name="psum_tr",
                         bufs=3 if "tr3" in cfg.probe else 4, space="PSUM"))
        psum_sc = ctx.enter_context(
            tc.tile_pool(name="psum_sc", bufs=1, space="PSUM"))
        # combine's transpose psum shares the scalar pool: it only runs
        # inside the clamped-correction branch, so single-buffering is free
        psum_cb = psum_sc

        BCW = min(512, dout)  # matmul free-dim limit (one PSUM bank, fp32)

        def broadcast_row(row_ap, out_sbuf, width):
            for c0 in range(0, width, BCW):
                w = min(BCW, width - c0)
                ps = psum_sc.tile([P, BCW], F32, name="ps_bc", tag="sc")
                nc.tensor.matmul(ps[:, :w], ones_1[:], row_ap[:, ds(c0, w)],
                                 start=True, stop=True)
                nc.any.tensor_copy(out_sbuf[:, ds(c0, w)], ps[:, :w])

        # ---- tiny setup: bias broadcast ----
        with tc.tile_pool(name="setup", bufs=1) as setup:
            brow = setup.tile([1, dout], F32)
            nc.sync.dma_start(brow[:], bias_d[None, :])
            broadcast_row(brow, bias_bc, dout)

        def phase1_chunk(ph1, bt, laa, ob, ic):
            isl = ds(ic * cfg.in_chunk, cfg.in_chunk)
            col = ob * IC + ic
            # dw raw = (B*m)^T-block @ A chunk, K=rank
            nmm = cfg.in_chunk // 512 if cfg.in_chunk >= 512 else 1
            mmw = cfg.in_chunk // nmm
            ps_dw = [psum_mm.tile([P, mmw], F32, name="ps_dw", tag="mm")
                     for _ in range(nmm)]
            for s in range(nmm):
                nc.tensor.matmul(
                    ps_dw[s][:], bt[:, ts(ob, P)],
                    laa[:, ds(ic * cfg.in_chunk + s * mmw, mmw)],
                    start=True, stop=True)
            ftile = ph1.tile([P, cfg.in_chunk], F32, name="ftile")
            dma_f = nc.gpsimd if "dmaspread" in cfg.probe else nc.sync
            dma_f.dma_start(ftile[:], fi_v[:, ob, isl])
            # 1 + gamma*F on gpsimd (1-input, keeps DVE free)
            nc.gpsimd.tensor_scalar(
                ftile[:], ftile[:], FISHER_GAMMA, 1.0,
                mybir.AluOpType.mult, mybir.AluOpType.add)
            if "norecip" not in cfg.probe:
                nc.vector.reciprocal(ftile[:], ftile[:])
            dwt = ph1.tile([P, cfg.in_chunk], BF16, name="dwt")
            for s in range(nmm):
                nc.vector.tensor_tensor(
                    dwt[:, ds(s * mmw, mmw)], ps_dw[s][:],
                    ftile[:, ds(s * mmw, mmw)], mybir.AluOpType.mult)
            if "nospill" not in cfg.probe:
                dma_s = nc.gpsimd if "dmaspread" in cfg.probe else nc.sync
                dma_s.dma_start(dwn_v[:, ob, isl], dwt[:])
            if "nosquare" not in cfg.probe and "nofastpath" in cfg.probe:
                # in-place square (spill DMA has read dwt by WAR ordering);
                # only accum_out matters
                nc.scalar.activation(
                    dwt[:], dwt[:], mybir.ActivationFunctionType.Square,
                    accum_out=stats_dw[:, col:col + 1])
            w0t = ph1.tile([P, cfg.in_chunk], F32, name="w0t")
            nc.sync.dma_start(w0t[:], w0_v[:, ob, isl])  # sync queue
            if "nosquare" not in cfg.probe:
                # ftile's gate values are dead after the dwt multiply
                nc.scalar.activation(
                    ftile[:], w0t[:], mybir.ActivationFunctionType.Square,
                    accum_out=stats_w0[:, col:col + 1])
            wat = ph1.tile([P, cfg.in_chunk], F32, name="wat")
            dma_w = nc.sync
            dma_w.dma_start(wat[:], wa_v[:, ob, isl])
            nc.vector.tensor_add(w0t[:], w0t[:], wat[:])
            if "nofastpath" not in cfg.probe:
                # wt gets (S + dw)^T; combine then only applies the
                # (factor-1)*dw^T correction, skipped when factor == 1
                nc.vector.tensor_add(w0t[:], w0t[:], dwt[:])
                if "nosquare" not in cfg.probe:
                    # in-place square emitted last: every dwt consumer
                    # (spill DMA, the add above) is ordered before this write
                    nc.scalar.activation(
                        dwt[:], dwt[:], mybir.ActivationFunctionType.Square,
                        accum_out=stats_dw[:, col:col + 1])
            # transpose S chunk into wt; dw chunk into spill DRAM
            for g in range(0, CPB, 4):
                gn = min(4, CPB - g)
                pt = psum_tr.tile([P, gn * P], F32, name="pt_s", tag="pt")
                for jj in range(gn):
                    kb = g + jj
                    nc.tensor.matmul(
                        pt[:, ts(jj, P)], w0t[:, ts(kb, P)],
                        identity[:], is_transpose=True,
                        start=(jj == 0), stop=(jj == gn - 1))
                kb0 = ic * CPB + g
                nc.any.tensor_copy(
                    wt[:, ds(kb0, gn), ts(ob, P)],
                    pt[:].rearrange("p (g q) -> p g q", g=gn))


        def phase1():
            with tc.tile_pool(name="ph1_small", bufs=1) as small:
                # B^T via strided DMA (tiny); fold m and alpha/r into it
                bt = small.tile([cfg.rank, dout], F32, name="bt")
                nc.sync.dma_start(bt[:], lb_d.rearrange("o r -> r o"))
                with tc.tile_pool(name="msetup", bufs=1) as msetup:
                    mrow = msetup.tile([1, dout], F32)
                    nc.sync.dma_start(mrow[:], m_d[None, :])
                    m_bc = msetup.tile([P, dout], F32)
                    broadcast_row(mrow, m_bc, dout)
                    nc.any.tensor_scalar_mul(m_bc[:], m_bc[:],
                                             LORA_ALPHA / R_RANK)
                    nc.vector.tensor_mul(bt[:], bt[:], m_bc[:cfg.rank, :])

                laa = small.tile([cfg.rank, din], F32, name="laa")
                nc.sync.dma_start(laa[:], la_d[:, :])

                ph1_bufs = 2 if "bufs2" in cfg.probe else 3
                with tc.tile_pool(name="ph1", bufs=ph1_bufs) as ph1:
                    for ob in range(OB):
                        for ic in range(IC):
                            phase1_chunk(ph1, bt, laa, ob, ic)

        def norms_and_factor():
            nc.vector.reduce_sum(stats2[:, 0:1], stats_dw[:],
                                 axis=mybir.AxisListType.X)
            nc.vector.reduce_sum(stats2[:, 1:2], stats_w0[:],
                                 axis=mybir.AxisListType.X)
            ps_tot = psum_sc.tile([P, 2], F32, name="ps_tot", tag="sc")
            nc.tensor.matmul(ps_tot[:], ones_full[:], stats2[:],
                             start=True, stop=True)
            cc_sb = const.tile([1, 2], F32)
            nc.any.tensor_copy(cc_sb[:], ps_tot[0:1, :])
            nc.sync.dma_start(cc_in_d[None, :], cc_sb[:])
            if variant == "nocc":
                nc.sync.dma_start(cc_out_d[None, :], cc_in_d[None, :])
            else:
                nc.gpsimd.collective_compute(
                    "AllReduce", mybir.AluOpType.add, replica_groups=groups,
                    ins=[cc_in_d[:]], outs=[cc_out_d[:]])
            nc.sync.dma_start(t1[:], cc_out_d[None, :])
            ps_f = psum_sc.tile([P, 2], F32, name="ps_f", tag="sc")
            nc.tensor.matmul(ps_f[:], ones_1[:], t1[:], start=True, stop=True)
            nc.any.tensor_copy(tot_bc[:], ps_f[:])
            # factor = min(1, sqrt(eta^2*sum_w02 / max(sum_dw2, 1e-16)))
            num = const.tile([P, 1], F32)
            nc.vector.tensor_scalar_mul(num[:], tot_bc[:, 1:2],
                                        ENERGY_ETA * ENERGY_ETA)
            den = const.tile([P, 1], F32)
            nc.vector.tensor_scalar_max(den[:], tot_bc[:, 0:1], 1e-16)
            rat = const.tile([P, 1], F32)
            nc.vector.reciprocal(den[:], den[:])
            nc.vector.tensor_tensor(rat[:], num[:], den[:],
                                    mybir.AluOpType.mult)
            nc.scalar.sqrt(fac[:], rat[:])
            nc.vector.tensor_scalar_min(fac[:], fac[:], 1.0)
            if "nofastpath" not in cfg.probe:
                # clamped <=> sum_dw2 > eta^2*sum_w02 AND sum_dw2 > eps^2
                f1 = const.tile([P, 1], F32)
                nc.vector.tensor_tensor(f1[:], tot_bc[:, 0:1], num[:],
                                        mybir.AluOpType.is_gt)
                f2 = const.tile([P, 1], F32)
                nc.vector.tensor_scalar(
                    f2[:], tot_bc[:, 0:1], 1e-16, None,
                    mybir.AluOpType.is_gt)
                nc.vector.tensor_mul(f1[:], f1[:], f2[:])
                nc.any.tensor_copy(flagi[:], f1[:])
                # correction scalar: factor - 1
                nc.vector.tensor_scalar_add(facm1[:], fac[:], -1.0)

        def combine_body(scal):
            with tc.tile_pool(name="comb", bufs=3) as comb:
                obs = (list(reversed(range(OB))) if "combrev" in cfg.probe
                       else list(range(OB)))
                for ob in obs:
                    for ic in range(IC):
                        isl = ds(ic * cfg.in_chunk, cfg.in_chunk)
                        dwn = comb.tile([P, cfg.in_chunk], BF16, name="dwn")
                        nc.sync.dma_start(dwn[:], dwn_v[:, ob, isl])
                        for g in range(0, CPB, 8):
                            gn = min(8, CPB - g)
                            ptb = psum_cb.tile([P, gn * P], BF16, name="pt_c",
                                               tag="sc")
                            for jj in range(gn):
                                nc.tensor.matmul(
                                    ptb[:, ts(jj, P)], dwn[:, ts(g + jj, P)],
                                    identity_bf[:], is_transpose=True,
                                    start=(jj == 0), stop=(jj == gn - 1))
                            kb0 = ic * CPB + g
                            nc.vector.scalar_tensor_tensor(
                                wt[:, ds(kb0, gn), ts(ob, P)],
                                ptb[:].rearrange("p (g q) -> p g q", g=gn),
                                scal[:],
                                wt[:, ds(kb0, gn), ts(ob, P)],
                                mybir.AluOpType.mult, mybir.AluOpType.add)

        def combine():
            if "nocombine" in cfg.probe:
                return
            if "nofastpath" in cfg.probe:
                combine_body(fac)
                return
            regs = nc.alloc_registers()
            nc.regs_load(regs, flagi[0:1, 0:1])
            cond = nc.snap(regs, donate=True)
            with tc.If(cond):
                combine_body(facm1)

        def phase2():
            XH = 2048 if din >= 4096 else din  # x staged in halves
            if "xh1024" in cfg.probe:
                XH = min(1024, din)
            with tc.tile_pool(name="ph2", bufs=2) as ph2, \
                    tc.tile_pool(name="ph2y", bufs=3) as ph2y:
                xt_bufs = 3 if "xt3" in cfg.probe else 2
                for mt in range(MT):
                    xt = ph2.tile([P, KB, P], mm_dt, name="xt", bufs=xt_bufs)
                    for h in range(0, din, XH):
                        xs = ph2.tile([P, XH], F32, name="xs")
                        nc.sync.dma_start(xs[:], x_d[ts(mt, P), ds(h, XH)])
                        if cfg.use_f32r and "trf32" not in cfg.probe:
                            xsr = ph2.tile([P, XH], F32R, name="xsr")
                            nc.scalar.copy(xsr[:], xs[:])
                            xs = xsr
                        for g in range(0, XH // P, 4):
                            gn = min(4, XH // P - g)
                            pt = psum_tr.tile([P, gn * P], F32, name="pt_x",
                                              tag="pt")
                            trr = cfg.use_f32r and "trf32" not in cfg.probe
                            for jj in range(gn):
                                xsb = xs[:, ts(g + jj, P)]
                                idn = identity[:]
                                ptb = pt[:, ts(jj, P)]
                                if trr:
                                    idn = identity_r[:]
                                    ptb = ptb.bitcast(F32R)
                                nc.tensor.matmul(
                                    ptb, xsb, idn,
                                    is_transpose=True,
                                    start=(jj == 0), stop=(jj == gn - 1))
                            nc.any.tensor_copy(
                                xt[:, ds(h // P + g, gn), :],
                                pt[:].rearrange("p (g q) -> p g q", g=gn))
                    for nt in range(NT):
                        osl = ds(nt * cfg.nfree, cfg.nfree)
                        ps_y = psum_mm.tile([P, cfg.nfree], F32, name="ps_y",
                                            tag="mm")
                        for kb in range(KB):
                            nc.tensor.matmul(
                                ps_y[:], xt[:, kb, :], wt[:, kb, osl],
                                start=(kb == 0), stop=(kb == KB - 1))
                        ys = ph2y.tile([P, cfg.nfree], F32, name="ys")
                        nc.vector.tensor_add(ys[:], ps_y[:], bias_bc[:, osl])
                        nc.sync.dma_start(y_d[ts(mt, P), osl], ys[:])

        if variant == "ph2":
            nc.vector.memset(wt[:], 0.01)
        else:
            phase1()
            norms_and_factor()
            combine()
        if variant != "ph1":
            phase2()

    nc.compile()
    return nc


_CACHED = {}


def _get_nc(key, cfg, variant="full"):
    if key not in _CACHED:
        _CACHED[key] = build_nc(cfg, variant)
    return _CACHED[key]


def make_in_maps(x, W0, W_acc, bias, lora_A, lora_B, m, fisher_mask, cfg: Cfg):
    xf = np.ascontiguousarray(x.reshape(-1, cfg.din))
    in_maps = []
    for c in range(cfg.n_cores):
        i, j = c // cfg.col_groups, c % cfg.col_groups
        rs = slice(i * cfg.tok, (i + 1) * cfg.tok)
        cs = slice(j * cfg.dout, (j + 1) * cfg.dout)
        in_maps.append({
            "x": xf[rs],
            "w0": np.ascontiguousarray(W0[cs]),
            "wacc": np.ascontiguousarray(W_acc[cs]),
            "fisher": np.ascontiguousarray(fisher_mask[cs]),
            "lora_a": np.ascontiguousarray(lora_A),
            "lora_b": np.ascontiguousarray(lora_B[cs]),
            "mvec": np.ascontiguousarray(m[cs]),
            "bias": np.ascontiguousarray(bias[cs]),
        })
    return in_maps


def assemble_out(results, cfg: Cfg, out_shape):
    tok_total = cfg.tok * cfg.row_groups
    y = np.empty((tok_total, cfg.dout * cfg.col_groups), np.float32)
    for c in range(cfg.n_cores):
        i, j = c // cfg.col_groups, c % cfg.col_groups
        y[i * cfg.tok:(i + 1) * cfg.tok,
          j * cfg.dout:(j + 1) * cfg.dout] = results[c]["y"]
    return y.reshape(out_shape)


def full_cfg():
    return Cfg(tok=B_FULL * S_FULL // ROW_GROUPS, din=D_IN,
               dout=D_OUT // COL_GROUPS)


def kernel(x, W0, W_acc, bias, lora_A, lora_B, m, fisher_mask):
    cfg = full_cfg()
    nc = _get_nc("full", cfg)
    in_maps = make_in_maps(x, W0, W_acc, bias, lora_A, lora_B, m,
                           fisher_mask, cfg)
    res = run_bass_kernel_spmd(nc, in_maps, core_ids=list(range(cfg.n_cores)))
    return assemble_out(res.results, cfg, (B_FULL, S_FULL, D_OUT))


# revision 34
# speedup vs baseline: 1.8469x; 1.8469x over previous
"""Trainium2 Bass kernel for nn_AeloruLayer (Hi-DoRA/Fisher-gated LoRA linear).

Computation (reference semantics, all fp32):
    dw     = (alpha/r) * (lora_B @ lora_A) * m[:, None] / (1 + gamma * fisher)
    factor = min(1, eta * ||W0||_F / max(||dw||_F, eps))   (branchless-equivalent)
    W_eff  = W0 + W_acc + factor * dw
    y      = x @ W_eff^T + bias

Sharding across 8 NeuronCores: 2-way over batch/tokens x 4-way over
out_features.  Each core computes a (TOK x DOUT) block of y; the host
concatenates.  The Frobenius norms need a global reduction: each core
reduces its shard, then a tiny AllReduce over the 4 cores that together
hold the full weight matrix (one batch group) combines them.

Device pipeline per core:
  Phase 1 (weights): stream W0/W_acc/fisher shard in [128 x IN_CHUNK]
    tiles; dw = (B*m @ A) / (1+gamma*F) via PE matmul (K=16) + DVE
    reciprocal/multiply; accumulate sum(dw^2), sum(W0^2) on ACT;
    S = W0+W_acc; PE-transpose S into the SBUF-resident W_eff^T buffer;
    PE-transpose dw and spill dw^T to DRAM (SBUF can't hold both S^T
    and dw^T).
  Sync: AllReduce([sum_dw2, sum_w02]) -> factor (branchless min/sqrt),
    broadcast to all partitions via a K=1 ones-matmul.
  Combine: stream dw^T back, W_eff^T += factor * dw^T.
  Phase 2 (tokens): per 128-token tile, PE-transpose x into x^T
    (contraction dim must sit on partitions), then float32r matmuls
    accumulate y tiles in PSUM; evict with fused bias add; DMA out.
"""

from contextlib import ExitStack

import numpy as np

import concourse.bass as bass
import concourse.mybir as mybir
import concourse.tile as tile
from concourse import bacc
from concourse.bass import ds, ts
from concourse.bass_utils import run_bass_kernel_spmd
from concourse.masks import make_identity

F32 = mybir.dt.float32
F32R = mybir.dt.float32r
P = 128

LORA_ALPHA = 16.0
R_RANK = 16
FISHER_GAMMA = 10.0
ENERGY_ETA = 0.15

# problem shapes (full)
B_FULL, S_FULL, D_IN, D_OUT = 4, 2048, 4096, 4096
N_CORES = 8
ROW_GROUPS = 2  # batch split
COL_GROUPS = 4  # out_features split


class Cfg:
    def __init__(self, tok, din, dout, rank=R_RANK, n_cores=N_CORES,
                 row_groups=ROW_GROUPS, col_groups=COL_GROUPS,
                 in_chunk=1024, nfree=512, use_f32r=True, probe=()):
        self.tok = tok          # tokens per core
        self.din = din          # contraction dim (full)
        self.dout = dout        # out features per core
        self.rank = rank
        self.n_cores = n_cores
        self.row_groups = row_groups
        self.col_groups = col_groups
        self.in_chunk = min(in_chunk, din)
        self.nfree = min(nfree, dout)
        self.use_f32r = use_f32r
        self.probe = set(probe)  # timing-probe ablations, see build_nc
        assert tok % P == 0 and din % P == 0 and dout % P == 0
        assert din % self.in_chunk == 0 and self.in_chunk % P == 0
        assert dout % self.nfree == 0


def build_nc(cfg: Cfg, variant="full"):
    """variant: "full" | "ph1" (weights pipeline only) | "ph2" (token
    matmul pipeline only, wt from memset) | "nocc" (full, collective
    replaced by a local DMA copy). Non-full variants are for timing
    bisection only."""
    nc = bacc.Bacc("TRN2", target_bir_lowering=False, debug=False,
                   num_devices=cfg.n_cores)
    tok, din, dout = cfg.tok, cfg.din, cfg.dout
    KB = din // P            # 128-blocks along contraction dim
    OB = dout // P           # 128-blocks along out dim
    IC = din // cfg.in_chunk
    CPB = cfg.in_chunk // P  # 128-blocks per phase-1 chunk
    MT = tok // P            # token tiles
    NT = dout // cfg.nfree   # out tiles in phase 2
    mm_dt = F32R if cfg.use_f32r else F32

    x_d = nc.dram_tensor("x", [tok, din], F32, kind="ExternalInput").ap()
    w0_d = nc.dram_tensor("w0", [dout, din], F32, kind="ExternalInput").ap()
    wa_d = nc.dram_tensor("wacc", [dout, din], F32, kind="ExternalInput").ap()
    fi_d = nc.dram_tensor("fisher", [dout, din], F32, kind="ExternalInput").ap()
    la_d = nc.dram_tensor("lora_a", [cfg.rank, din], F32, kind="ExternalInput").ap()
    lb_d = nc.dram_tensor("lora_b", [dout, cfg.rank], F32, kind="ExternalInput").ap()
    m_d = nc.dram_tensor("mvec", [dout], F32, kind="ExternalInput").ap()
    bias_d = nc.dram_tensor("bias", [dout], F32, kind="ExternalInput").ap()
    y_d = nc.dram_tensor("y", [tok, dout], F32, kind="ExternalOutput").ap()
    BF16 = mybir.dt.bfloat16
    dwn_d = nc.dram_tensor("dw_spill", [dout, din], BF16).ap()
    cc_in_d = nc.dram_tensor("cc_in", [2], F32).ap()
    cc_out_d = nc.dram_tensor("cc_out", [2], F32).ap()

    # out-block views: row (ob*128 + p) -> [p, ob, :]
    w0_v = w0_d.rearrange("(ob p) i -> p ob i", p=P)
    wa_v = wa_d.rearrange("(ob p) i -> p ob i", p=P)
    fi_v = fi_d.rearrange("(ob p) i -> p ob i", p=P)
    dwn_v = dwn_d.rearrange("(ob p) i -> p ob i", p=P)

    groups = [[i * cfg.col_groups + j for j in range(cfg.col_groups)]
              for i in range(cfg.row_groups)]

    with tile.TileContext(nc) as tc, ExitStack() as ctx:
        const = ctx.enter_context(tc.tile_pool(name="const", bufs=1))
        identity = const.tile([P, P], F32)
        make_identity(nc, identity)
        ones_full = const.tile([P, P], F32)
        nc.vector.memset(ones_full[:], 1.0)
        ones_1 = const.tile([1, P], F32)
        nc.vector.memset(ones_1[:], 1.0)
        identity_bf = const.tile([P, P], BF16)
        make_identity(nc, identity_bf)
        identity_r = const.tile([P, P], F32R)
        nc.vector.tensor_copy(identity_r[:], identity[:])
        stats_dw = const.tile([P, OB * IC], F32)
        nc.vector.memset(stats_dw[:], 0.0)
        stats_w0 = const.tile([P, OB * IC], F32)
        nc.vector.memset(stats_w0[:], 0.0)
        stats2 = const.tile([P, 2], F32)
        tot_bc = const.tile([P, 2], F32)
        fac = const.tile([P, 1], F32)
        bias_bc = const.tile([P, dout], F32)
        t1 = const.tile([1, 2], F32)
        flagi = const.tile([P, 1], mybir.dt.int32)
        facm1 = const.tile([P, 1], F32)

        # W_eff^T, resident: [in-part, in-block, out].  Typed float32r so
        # every write rounds, as the fp32r matmul verifier requires.
        wt = const.tile([P, KB, dout], mm_dt, name="w_eff_t")

        psum_mm = ctx.enter_context(
            tc.tile_pool(name="psum_mm",
                         bufs=3 if "mm3" in cfg.probe else 2, space="PSUM"))
        psum_tr = ctx.enter_context(
            tc.tile_pool(name="psum_tr",
                         bufs=3 if "tr3" in cfg.probe else 4, space="PSUM"))
        psum_sc = ctx.enter_context(
            tc.tile_pool(name="psum_sc", bufs=1, space="PSUM"))
        # combine's transpose psum shares the scalar pool: it only runs
        # inside the clamped-correction branch, so single-buffering is free
        psum_cb = psum_sc

        BCW = min(512, dout)  # matmul free-dim limit (one PSUM bank, fp32)

        def broadcast_row(row_ap, out_sbuf, width):
            for c0 in range(0, width, BCW):
                w = min(BCW, width - c0)
                ps = psum_sc.tile([P, BCW], F32, name="ps_bc", tag="sc")
                nc.tensor.matmul(ps[:, :w], ones_1[:], row_ap[:, ds(c0, w)],
                                 start=True, stop=True)
                nc.any.tensor_copy(out_sbuf[:, ds(c0, w)], ps[:, :w])

        # ---- tiny setup: bias broadcast ----
        with tc.tile_pool(name="setup", bufs=1) as setup:
            brow = setup.tile([1, dout], F32)
            nc.sync.dma_start(brow[:], bias_d[None, :])
            broadcast_row(brow, bias_bc, dout)

        def phase1_chunk(ph1, bt, laa, ob, ic):
            isl = ds(ic * cfg.in_chunk, cfg.in_chunk)
            col = ob * IC + ic
            # dw raw = (B*m)^T-block @ A chunk, K=rank
            nmm = cfg.in_chunk // 512 if cfg.in_chunk >= 512 else 1
            mmw = cfg.in_chunk // nmm
            ps_dw = [psum_mm.tile([P, mmw], F32, name="ps_dw", tag="mm")
                     for _ in range(nmm)]
            for s in range(nmm):
                nc.tensor.matmul(
                    ps_dw[s][:], bt[:, ts(ob, P)],
                    laa[:, ds(ic * cfg.in_chunk + s * mmw, mmw)],
                    start=True, stop=True)
            ftile = ph1.tile([P, cfg.in_chunk], F32, name="ftile")
            dma_f = nc.gpsimd if "dmaspread" in cfg.probe else nc.sync
            dma_f.dma_start(ftile[:], fi_v[:, ob, isl])
            # 1 + gamma*F on gpsimd (1-input, keeps DVE free)
            nc.gpsimd.tensor_scalar(
                ftile[:], ftile[:], FISHER_GAMMA, 1.0,
                mybir.AluOpType.mult, mybir.AluOpType.add)
            if "norecip" not in cfg.probe:
                nc.vector.reciprocal(ftile[:], ftile[:])
            dwt = ph1.tile([P, cfg.in_chunk], BF16, name="dwt")
            for s in range(nmm):
                nc.vector.tensor_tensor(
                    dwt[:, ds(s * mmw, mmw)], ps_dw[s][:],
                    ftile[:, ds(s * mmw, mmw)], mybir.AluOpType.mult)
            if "nospill" not in cfg.probe:
                dma_s = nc.gpsimd if "dmaspread" in cfg.probe else nc.sync
                dma_s.dma_start(dwn_v[:, ob, isl], dwt[:])
            if "nosquare" not in cfg.probe and "nofastpath" in cfg.probe:
                # in-place square (spill DMA has read dwt by WAR ordering);
                # only accum_out matters
                nc.scalar.activation(
                    dwt[:], dwt[:], mybir.ActivationFunctionType.Square,
                    accum_out=stats_dw[:, col:col + 1])
            w0t = ph1.tile([P, cfg.in_chunk], F32, name="w0t")
            nc.sync.dma_start(w0t[:], w0_v[:, ob, isl])  # sync queue
            if "nosquare" not in cfg.probe:
                # ftile's gate values are dead after the dwt multiply
                nc.scalar.activation(
                    ftile[:], w0t[:], mybir.ActivationFunctionType.Square,
                    accum_out=stats_w0[:, col:col + 1])
            wat = ph1.tile([P, cfg.in_chunk], F32, name="wat")
            dma_w = nc.sync
            dma_w.dma_start(wat[:], wa_v[:, ob, isl])
            nc.vector.tensor_add(w0t[:], w0t[:], wat[:])
            if "nofastpath" not in cfg.probe:
                # wt gets (S + dw)^T; combine then only applies the
                # (factor-1)*dw^T correction, skipped when factor == 1
                nc.vector.tensor_add(w0t[:], w0t[:], dwt[:])
                if "nosquare" not in cfg.probe:
                    # in-place square emitted last: every dwt consumer
                    # (spill DMA, the add above) is ordered before this write
                    nc.scalar.activation(
                        dwt[:], dwt[:], mybir.ActivationFunctionType.Square,
                        accum_out=stats_dw[:, col:col + 1])
            # transpose S chunk into wt; dw chunk into spill DRAM
            for g in range(0, CPB, 4):
                gn = min(4, CPB - g)
                pt = psum_tr.tile([P, gn * P], F32, name="pt_s", tag="pt")
                for jj in range(gn):
                    kb = g + jj
                    nc.tensor.matmul(
                        pt[:, ts(jj, P)], w0t[:, ts(kb, P)],
                        identity[:], is_transpose=True,
                        start=(jj == 0), stop=(jj == gn - 1))
                kb0 = ic * CPB + g
                nc.any.tensor_copy(
                    wt[:, ds(kb0, gn), ts(ob, P)],
                    pt[:].rearrange("p (g q) -> p g q", g=gn))


        def phase1():
            with tc.tile_pool(name="ph1_small", bufs=1) as small:
                # B^T via strided DMA (tiny); fold m and alpha/r into it
                bt = small.tile([cfg.rank, dout], F32, name="bt")
                nc.sync.dma_start(bt[:], lb_d.rearrange("o r -> r o"))
                with tc.tile_pool(name="msetup", bufs=1) as msetup:
                    mrow = msetup.tile([1, dout], F32)
                    nc.sync.dma_start(mrow[:], m_d[None, :])
                    m_bc = msetup.tile([P, dout], F32)
                    broadcast_row(mrow, m_bc, dout)
                    nc.any.tensor_scalar_mul(m_bc[:], m_bc[:],
                                             LORA_ALPHA / R_RANK)
                    nc.vector.tensor_mul(bt[:], bt[:], m_bc[:cfg.rank, :])

                laa = small.tile([cfg.rank, din], F32, name="laa")
                nc.sync.dma_start(laa[:], la_d[:, :])

                ph1_bufs = 2 if "bufs2" in cfg.probe else 3
                with tc.tile_pool(name="ph1", bufs=ph1_bufs) as ph1:
                    for ob in range(OB):
                        for ic in range(IC):
                            phase1_chunk(ph1, bt, laa, ob, ic)

        def norms_and_factor():
            nc.vector.reduce_sum(stats2[:, 0:1], stats_dw[:],
                                 axis=mybir.AxisListType.X)
            nc.vector.reduce_sum(stats2[:, 1:2], stats_w0[:],
                                 axis=mybir.AxisListType.X)
            ps_tot = psum_sc.tile([P, 2], F32, name="ps_tot", tag="sc")
            nc.tensor.matmul(ps_tot[:], ones_full[:], stats2[:],
                             start=True, stop=True)
            cc_sb = const.tile([1, 2], F32)
            nc.any.tensor_copy(cc_sb[:], ps_tot[0:1, :])
            nc.sync.dma_start(cc_in_d[None, :], cc_sb[:])
            if variant == "nocc":
                nc.sync.dma_start(cc_out_d[None, :], cc_in_d[None, :])
            else:
                nc.gpsimd.collective_compute(
                    "AllReduce", mybir.AluOpType.add, replica_groups=groups,
                    ins=[cc_in_d[:]], outs=[cc_out_d[:]])
            nc.sync.dma_start(t1[:], cc_out_d[None, :])
            ps_f = psum_sc.tile([P, 2], F32, name="ps_f", tag="sc")
            nc.tensor.matmul(ps_f[:], ones_1[:], t1[:], start=True, stop=True)
            nc.any.tensor_copy(tot_bc[:], ps_f[:])
            # factor = min(1, sqrt(eta^2*sum_w02 / max(sum_dw2, 1e-16)))
            num = const.tile([P, 1], F32)
            nc.vector.tensor_scalar_mul(num[:], tot_bc[:, 1:2],
                                        ENERGY_ETA * ENERGY_ETA)
            den = const.tile([P, 1], F32)
            nc.vector.tensor_scalar_max(den[:], tot_bc[:, 0:1], 1e-16)
            rat = const.tile([P, 1], F32)
            nc.vector.reciprocal(den[:], den[:])
            nc.vector.tensor_tensor(rat[:], num[:], den[:],
                                    mybir.AluOpType.mult)
            nc.scalar.sqrt(fac[:], rat[:])
            nc.vector.tensor_scalar_min(fac[:], fac[:], 1.0)
            if "nofastpath" not in cfg.probe:
                # clamped <=> sum_dw2 > eta^2*sum_w02 AND sum_dw2 > eps^2
                f1 = const.tile([P, 1], F32)
                nc.vector.tensor_tensor(f1[:], tot_bc[:, 0:1], num[:],
                                        mybir.AluOpType.is_gt)
                f2 = const.tile([P, 1], F32)
                nc.vector.tensor_scalar(
                    f2[:], tot_bc[:, 0:1], 1e-16, None,
                    mybir.AluOpType.is_gt)
                nc.vector.tensor_mul(f1[:], f1[:], f2[:])
                nc.any.tensor_copy(flagi[:], f1[:])
                # correction scalar: factor - 1
                nc.vector.tensor_scalar_add(facm1[:], fac[:], -1.0)

        def combine_body(scal):
            with tc.tile_pool(name="comb", bufs=3) as comb:
                obs = (list(reversed(range(OB))) if "combrev" in cfg.probe
                       else list(range(OB)))
                for ob in obs:
                    for ic in range(IC):
                        isl = ds(ic * cfg.in_chunk, cfg.in_chunk)
                        dwn = comb.tile([P, cfg.in_chunk], BF16, name="dwn")
                        nc.sync.dma_start(dwn[:], dwn_v[:, ob, isl])
                        for g in range(0, CPB, 8):
                            gn = min(8, CPB - g)
                            ptb = psum_cb.tile([P, gn * P], BF16, name="pt_c",
                                               tag="sc")
                            for jj in range(gn):
                                nc.tensor.matmul(
                                    ptb[:, ts(jj, P)], dwn[:, ts(g + jj, P)],
                                    identity_bf[:], is_transpose=True,
                                    start=(jj == 0), stop=(jj == gn - 1))
                            kb0 = ic * CPB + g
                            nc.vector.scalar_tensor_tensor(
                                wt[:, ds(kb0, gn), ts(ob, P)],
                                ptb[:].rearrange("p (g q) -> p g q", g=gn),
                                scal[:],
                                wt[:, ds(kb0, gn), ts(ob, P)],
                                mybir.AluOpType.mult, mybir.AluOpType.add)

        def combine():
            if "nocombine" in cfg.probe:
                return
            if "nofastpath" in cfg.probe:
                combine_body(fac)
                return
            regs = nc.alloc_registers()
            nc.regs_load(regs, flagi[0:1, 0:1])
            cond = nc.snap(regs, donate=True)
            with tc.If(cond):
                combine_body(facm1)

        def phase2():
            # x staged in 1024-wide quarters: smaller DMA granules pipeline
            # better with the 4-deep transpose psum pool (cost model: -7.5us)
            XH = 1024 if din >= 2048 else din
            if "xh2048" in cfg.probe:
                XH = min(2048, din)
            with tc.tile_pool(name="ph2", bufs=2) as ph2, \
                    tc.tile_pool(name="ph2y", bufs=3) as ph2y:
                xt_bufs = 3 if "xt3" in cfg.probe else 2
                for mt in range(MT):
                    xt = ph2.tile([P, KB, P], mm_dt, name="xt", bufs=xt_bufs)
                    for h in range(0, din, XH):
                        xs = ph2.tile([P, XH], F32, name="xs")
                        nc.sync.dma_start(xs[:], x_d[ts(mt, P), ds(h, XH)])
                        if cfg.use_f32r and "trf32" not in cfg.probe:
                            xsr = ph2.tile([P, XH], F32R, name="xsr")
                            nc.scalar.copy(xsr[:], xs[:])
                            xs = xsr
                        for g in range(0, XH // P, 4):
                            gn = min(4, XH // P - g)
                            pt = psum_tr.tile([P, gn * P], F32, name="pt_x",
                                              tag="pt")
                            trr = cfg.use_f32r and "trf32" not in cfg.probe
                            for jj in range(gn):
                                xsb = xs[:, ts(g + jj, P)]
                                idn = identity[:]
                                ptb = pt[:, ts(jj, P)]
                                if trr:
                                    idn = identity_r[:]
                                    ptb = ptb.bitcast(F32R)
                                nc.tensor.matmul(
                                    ptb, xsb, idn,
                                    is_transpose=True,
                                    start=(jj == 0), stop=(jj == gn - 1))
                            nc.any.tensor_copy(
                                xt[:, ds(h // P + g, gn), :],
                                pt[:].rearrange("p (g q) -> p g q", g=gn))
                    for nt in range(NT):
                        osl = ds(nt * cfg.nfree, cfg.nfree)
                        ps_y = psum_mm.tile([P, cfg.nfree], F32, name="ps_y",
                                            tag="mm")
                        for kb in range(KB):
                            nc.tensor.matmul(
                                ps_y[:], xt[:, kb, :], wt[:, kb, osl],
                                start=(kb == 0), stop=(kb == KB - 1))
                        ys = ph2y.tile([P, cfg.nfree], F32, name="ys")
                        nc.vector.tensor_add(ys[:], ps_y[:], bias_bc[:, osl])
                        nc.sync.dma_start(y_d[ts(mt, P), osl], ys[:])

        if variant == "ph2":
            nc.vector.memset(wt[:], 0.01)
        else:
            phase1()
            norms_and_factor()
            combine()
        if variant != "ph1":
            phase2()

    nc.compile()
    return nc


_CACHED = {}


def _get_nc(key, cfg, variant="full"):
    if key not in _CACHED:
        _CACHED[key] = build_nc(cfg, variant)
    return _CACHED[key]


def make_in_maps(x, W0, W_acc, bias, lora_A, lora_B, m, fisher_mask, cfg: Cfg):
    xf = np.ascontiguousarray(x.reshape(-1, cfg.din))
    in_maps = []
    for c in range(cfg.n_cores):
        i, j = c // cfg.col_groups, c % cfg.col_groups
        rs = slice(i * cfg.tok, (i + 1) * cfg.tok)
        cs = slice(j * cfg.dout, (j + 1) * cfg.dout)
        in_maps.append({
            "x": xf[rs],
            "w0": np.ascontiguousarray(W0[cs]),
            "wacc": np.ascontiguousarray(W_acc[cs]),
            "fisher": np.ascontiguousarray(fisher_mask[cs]),
            "lora_a": np.ascontiguousarray(lora_A),
            "lora_b": np.ascontiguousarray(lora_B[cs]),
            "mvec": np.ascontiguousarray(m[cs]),
            "bias": np.ascontiguousarray(bias[cs]),
        })
    return in_maps


def assemble_out(results, cfg: Cfg, out_shape):
    tok_total = cfg.tok * cfg.row_groups
    y = np.empty((tok_total, cfg.dout * cfg.col_groups), np.float32)
    for c in range(cfg.n_cores):
        i, j = c // cfg.col_groups, c % cfg.col_groups
        y[i * cfg.tok:(i + 1) * cfg.tok,
          j * cfg.dout:(j + 1) * cfg.dout] = results[c]["y"]
    return y.reshape(out_shape)


def full_cfg():
    return Cfg(tok=B_FULL * S_FULL // ROW_GROUPS, din=D_IN,
               dout=D_OUT // COL_GROUPS)


def kernel(x, W0, W_acc, bias, lora_A, lora_B, m, fisher_mask):
    cfg = full_cfg()
    nc = _get_nc("full", cfg)
    in_maps = make_in_maps(x, W0, W_acc, bias, lora_A, lora_B, m,
                           fisher_mask, cfg)
    res = run_bass_kernel_spmd(nc, in_maps, core_ids=list(range(cfg.n_cores)))
    return assemble_out(res.results, cfg, (B_FULL, S_FULL, D_OUT))
